# revision 22
# baseline (speedup 1.0000x reference)
"""Trainium2 Bass kernel for nn_AdapterModule_46050639348236.

Math: out = LayerNorm(silu(x @ W^T)) * gamma + beta, where
W = sum_i softmax(gate)_i * kron(B_i, A_i)  -- a [128, 2048] dense matrix.
(The per-factor einsum 'jk,bkl,li->bji' with row-major reshape of x is
exactly (kron(B_i, A_i) @ x_token); summing the 10 factors collapses the
whole adapter into one GEMM.)

Distribution: pure data-parallel over tokens, 8 NeuronCores, no
collectives. The host packs x (cast to bf16) into a layout where each
512-token chunk is one DMA with 16 KiB-contiguous per-partition reads and
the contraction axis on SBUF partitions; the device runs a K=2048 bf16
GEMM accumulating in f32 PSUM [128f, 512t], silu on ScalarE (bf16 out),
PE-mode transpose to [t, f], LayerNorm stats via bn_stats/bn_aggr on
VectorE, and streams the normalized bf16 result back in a
partition-contiguous layout the host unpermutes.

Scheduling: software pipeline with a 1-chunk lag (chunk k's GEMM streams
on the PE while chunk k-1's transposes/stats run) so the PE never idles
waiting on the silu -- an idle PE drops the HAM clock to half rate. The
LayerNorm sqrt is batched per group of chunks so the ScalarE activation
table doesn't thrash between Silu and Sqrt on every chunk (~1.5 us per
swap), and the normalization applies are spread one chunk per iteration
so they never form a long serial burst in the VectorE stream.
"""

import os
import sys

import numpy as np

# --- problem constants (hardcoded; kernel.py must be self-contained) ---
N_CORES = 8
T_TOTAL = 4 * 8192
T_CORE = T_TOTAL // N_CORES  # 4096
N_IN = 2048
M_OUT = 128
CHUNK = 512                  # tokens per device-side pipeline step
NJ = CHUNK // 128            # 4 token sub-tiles per chunk
N_CHUNKS = T_CORE // CHUNK   # 8
GROUPS = [4, 2, 1, 1]        # chunks per batched-sqrt group
KC = N_IN // 128             # 16 contraction chunks
LN_EPS = 1e-5

_NC_CACHE = {}


def _ensure_paths():
    for p in ("/opt/trn_rl_repo",):
        if os.path.isdir(p) and p not in sys.path:
            sys.path.append(p)


def _install_ntff_hook():
    """Provide antenv.axon_hooks so trace=True can capture NTFF profiles
    under axon. Harmless if tracing is never requested."""
    import types

    try:
        import antenv
    except ImportError:
        return
    if "antenv.axon_hooks" in sys.modules:
        return
    mod = types.ModuleType("antenv.axon_hooks")
    state = {"hook": None, "tried": False}

    def get_axon_ntff_profile_hook():
        if state["hook"] is None and not state["tried"]:
            state["tried"] = True
            try:
                from trn_agent_boot.trn_boot import _ntff_profile_via_ctypes

                state["hook"] = _ntff_profile_via_ctypes("/opt/axon/libaxon_pjrt.so")
            except Exception:
                state["hook"] = None
        return state["hook"]

    mod.get_axon_ntff_profile_hook = get_axon_ntff_profile_hook
    mod.set_axon_ntff_profile_hook = lambda h: state.update(hook=h, tried=True)
    sys.modules["antenv.axon_hooks"] = mod
    antenv.axon_hooks = mod


def _build_nc(trivial_affine):
    import concourse.bass as bass  # noqa: F401
    import concourse.mybir as mybir
    import concourse.tile as tile
    from concourse import bacc

    F32 = mybir.dt.float32
    BF16 = mybir.dt.bfloat16
    AF = mybir.ActivationFunctionType
    ALU = mybir.AluOpType

    assert sum(GROUPS) == N_CHUNKS
    # chunk -> (group index, offset in group); group -> first chunk
    chunk_group = {}
    group_start = []
    c0 = 0
    for gi, gs in enumerate(GROUPS):
        group_start.append(c0)
        for kk in range(gs):
            chunk_group[c0 + kk] = (gi, kk)
        c0 += gs

    nc = bacc.Bacc()
    # host-packed: xt[k, p, h, t] = x[k*512 + t, h*128 + p], bf16
    xt_ext = nc.declare_dram_parameter(
        "xt", [N_CHUNKS, 128, KC, CHUNK], BF16, isOutput=False
    )
    # wt[p, h, f] = W[f, h*128 + p], bf16
    wt_ext = nc.declare_dram_parameter("wt", [128, KC, M_OUT], BF16, isOutput=False)
    id_ext = nc.declare_dram_parameter("ident", [128, 128], F32, isOutput=False)
    if not trivial_affine:
        gb_ext = nc.declare_dram_parameter("gb", [2, 128, 128], F32, isOutput=False)
    # out[k, p, j, f] = result[k*512 + j*128 + p, f]
    out_ext = nc.declare_dram_parameter(
        "out", [N_CHUNKS, 128, NJ, M_OUT], BF16, isOutput=True
    )

    with tile.TileContext(nc) as tc:
        with (
            tc.tile_pool(name="const", bufs=1) as constp,
            tc.tile_pool(name="xin", bufs=4) as xin,
            tc.tile_pool(name="work", bufs=4) as work,
            tc.tile_pool(name="outp", bufs=4) as outpool,
            tc.tile_pool(name="stats", bufs=2) as statsp,
            tc.tile_pool(name="pacc", bufs=2, space="PSUM") as pacc,
            tc.tile_pool(name="ptp", bufs=6, space="PSUM") as ptp,
        ):
            # Weights first (small, needed by the very first matmul), then
            # chunk 0's input quartered so the first GEMM starts after ~1/4
            # of the transfer instead of the full 2 MiB.
            wtt = constp.tile([128, KC, M_OUT], BF16)
            nc.sync.dma_start(out=wtt, in_=wt_ext[:, :, :])
            x0s = []
            for q in range(4):
                x0q = xin.tile([128, 4, CHUNK], BF16, tag=f"x0_{q}", name=f"x0_{q}")
                nc.sync.dma_start(out=x0q, in_=xt_ext[0][:, 4 * q:4 * q + 4, :])
                x0s.append(x0q)
            ident = constp.tile([128, 128], F32)
            nc.sync.dma_start(out=ident, in_=id_ext[:, :])
            if not trivial_affine:
                gbt = constp.tile([128, 2, 128], F32)
                nc.sync.dma_start(out=gbt, in_=gb_ext.rearrange("g p f -> p g f"))
            epst = constp.tile([128, 1], F32)
            nc.vector.memset(epst, float(LN_EPS))

            acts = {}    # chunk -> silu output tile
            tps = {}     # chunk -> transposed [t, f] PSUM tile
            mvgs = {}    # group -> mean/var tile
            rstds = {}   # group -> rstd tile
            apply_q = []  # chunks whose rstd is ready, awaiting apply

            def stats_stage(k):
                """Transposes + LN stats for chunk k; batched sqrt at group
                end (ScalarE table loads stay rare)."""
                g, kk = chunk_group[k]
                gs = GROUPS[g]
                if kk == 0:
                    mvgs[g] = statsp.tile(
                        [128, gs * NJ, 2], F32, tag="mv", name=f"mvg{g}"
                    )
                mvg = mvgs[g]
                tp = ptp.tile([128, CHUNK], F32, tag="tp", name=f"tp{k}")
                tps[k] = tp
                act = acts.pop(k)
                # All transposes first, then all stats: Tile's bank-level dep
                # tracking serializes same-bank PE-writes against DVE-reads in
                # emission order, so interleaving would stall each transpose
                # behind the previous bn_stats (~390 ns x 3 per chunk).
                for j in range(NJ):
                    fsl = slice(j * 128, (j + 1) * 128)
                    nc.tensor.transpose(tp[:, fsl], act[:, fsl], ident)
                for j in range(NJ):
                    fsl = slice(j * 128, (j + 1) * 128)
                    st = statsp.tile([128, 6], F32, tag="st")
                    nc.vector.bn_stats(out=st, in_=tp[:, fsl])
                    nc.vector.bn_aggr(out=mvg[:, kk * NJ + j, :], in_=st)

                if kk == gs - 1:
                    rstd = statsp.tile(
                        [128, gs * NJ], F32, tag="rstd", name=f"rstd{g}"
                    )
                    nc.scalar.activation(
                        out=rstd, in_=mvg[:, :, 1], func=AF.Sqrt, bias=epst
                    )
                    nc.vector.reciprocal(out=rstd, in_=rstd)
                    # Dummy op to pull the Silu-table reload off the critical
                    # path: it runs while the PE is still on the next GEMM,
                    # so the next real silu doesn't wait ~1.3us for the table.
                    # Reading rstd anchors it AFTER the sqrt — with a
                    # constant input the scheduler hoists it to kernel start.
                    dummy = statsp.tile([128, 1], F32, tag="dummy")
                    nc.scalar.activation(out=dummy, in_=rstd[:, 0:1], func=AF.Silu)
                    rstds[g] = rstd
                    apply_q.extend(group_start[g] + i for i in range(gs))

            def apply_stage(k2):
                """Normalize chunk k2 from its PSUM transpose and DMA out."""
                g, kk2 = chunk_group[k2]
                mvg, rstd = mvgs[g], rstds[g]
                outsb = outpool.tile([128, NJ, 128], BF16, tag="outsb")
                for j in range(NJ):
                    fsl = slice(j * 128, (j + 1) * 128)
                    c = kk2 * NJ + j
                    if trivial_affine:
                        nc.vector.tensor_scalar(
                            out=outsb[:, j, :], in0=tps[k2][:, fsl],
                            scalar1=mvg[:, c, 0:1], scalar2=rstd[:, c:c + 1],
                            op0=ALU.subtract, op1=ALU.mult,
                        )
                    else:
                        z = statsp.tile([128, 128], F32, tag="z")
                        nc.vector.tensor_scalar(
                            out=z, in0=tps[k2][:, fsl],
                            scalar1=mvg[:, c, 0:1], scalar2=rstd[:, c:c + 1],
                            op0=ALU.subtract, op1=ALU.mult,
                        )
                        nc.vector.tensor_mul(out=z, in0=z, in1=gbt[:, 0, :])
                        nc.vector.tensor_add(
                            out=outsb[:, j, :], in0=z, in1=gbt[:, 1, :]
                        )
                tps.pop(k2)
                nc.sync.dma_start(out=out_ext[k2], in_=outsb)

            LAG = 2  # chunks of slack between a GEMM and its epilogue
            k = 0
            while k < N_CHUNKS + LAG or apply_q:
                if k < N_CHUNKS:
                    if k == 0:
                        rhs_of = lambda c: x0s[c // 4][:, c % 4, :]
                    else:
                        xtt = xin.tile([128, KC, CHUNK], BF16, tag="xt")
                        nc.sync.dma_start(out=xtt, in_=xt_ext[k])
                        rhs_of = lambda c: xtt[:, c, :]
                    acc = pacc.tile([128, CHUNK], F32)  # [f, t] in PSUM
                    for c in range(KC):
                        nc.tensor.matmul(
                            acc, lhsT=wtt[:, c, :], rhs=rhs_of(c),
                            start=(c == 0), stop=(c == KC - 1),
                        )
                    act = work.tile([128, CHUNK], F32, tag="act")
                    nc.scalar.activation(out=act, in_=acc, func=AF.Silu)
                    acts[k] = act
                if LAG <= k < N_CHUNKS + LAG:
                    stats_stage(k - LAG)
                if apply_q:
                    apply_stage(apply_q.pop(0))
                if apply_q and k >= N_CHUNKS - 1:
                    # GEMMs are winding down: drain applies faster so the
                    # normalization tail doesn't extend past the last stats.
                    apply_stage(apply_q.pop(0))
                k += 1

    nc.compile()
    return nc


def _host_weights(A_factors, B_factors, gate):
    gate = np.asarray(gate, dtype=np.float64)
    e = np.exp(gate - gate.max())
    alpha = e / e.sum()
    W = np.zeros((M_OUT, N_IN), dtype=np.float64)
    for i, (a, b) in enumerate(zip(A_factors, B_factors)):
        W += alpha[i] * np.kron(
            np.asarray(b, dtype=np.float64), np.asarray(a, dtype=np.float64)
        )
    return W  # [128, 2048] float64


def kernel(x, A_factors, B_factors, gate, ln_gamma, ln_beta):
    _ensure_paths()
    _install_ntff_hook()
    import ml_dtypes
    from concourse.bass_utils import run_bass_kernel_spmd

    BF = ml_dtypes.bfloat16

    x = np.asarray(x, dtype=np.float32)
    gamma = np.asarray(ln_gamma, dtype=np.float32).reshape(M_OUT)
    beta = np.asarray(ln_beta, dtype=np.float32).reshape(M_OUT)
    trivial = bool(np.all(gamma == 1.0) and np.all(beta == 0.0))

    if trivial not in _NC_CACHE:
        _NC_CACHE[trivial] = _build_nc(trivial)
    nc = _NC_CACHE[trivial]

    W = _host_weights(A_factors, B_factors, gate)  # [128, 2048] f64
    # wt[p, h, f] = W[f, h*128+p]
    wt = np.ascontiguousarray(
        W.T.reshape(KC, 128, M_OUT).transpose(1, 0, 2)
    ).astype(BF)
    ident = np.eye(128, dtype=np.float32)

    in_maps = []
    xs = x.reshape(N_CORES, N_CHUNKS, CHUNK, KC, 128)  # [c, k, t, h, p]
    for c in range(N_CORES):
        # xt[k, p, h, t] = x[c, k, t, h, p]
        xt = np.ascontiguousarray(xs[c].transpose(0, 3, 2, 1)).astype(BF)
        m = {"xt": xt, "wt": wt, "ident": ident}
        if not trivial:
            m["gb"] = np.ascontiguousarray(
                np.stack([
                    np.broadcast_to(gamma.reshape(1, M_OUT), (128, M_OUT)),
                    np.broadcast_to(beta.reshape(1, M_OUT), (128, M_OUT)),
                ])
            )
        in_maps.append(m)

    trace = bool(int(os.environ.get("ADAPTER_TRACE", "0")))
    kwargs = {}
    if trace:
        kwargs["trace"] = True
        tdir = os.environ.get("ADAPTER_TRACE_DIR")
        if tdir:
            os.makedirs(tdir, exist_ok=True)
            kwargs["tmpdir"] = tdir

    res = None
    last_err = None
    for _attempt in range(3):
        try:
            res = run_bass_kernel_spmd(
                nc, in_maps, core_ids=list(range(N_CORES)), **kwargs
            )
            break
        except Exception as e:  # transient device wedge -> retry
            last_err = e
    if res is None:
        raise last_err
    if trace:
        print(f"HW exec time: {res.exec_time_ns} ns")

    # out[k, p, j, f] -> tokens k*512 + j*128 + p
    outs = [
        res.results[c]["out"].astype(np.float32)
        .transpose(0, 2, 1, 3).reshape(T_CORE, M_OUT)
        for c in range(N_CORES)
    ]
    out = np.concatenate(outs, axis=0)
    return out.reshape(4, 8192, M_OUT).astype(np.float32)


# revision 23
# speedup vs baseline: 1.1523x; 1.1523x over previous
"""Trainium2 Bass kernel for nn_AdapterModule_46050639348236.

Math: out = LayerNorm(silu(x @ W^T)) * gamma + beta, where
W = sum_i softmax(gate)_i * kron(B_i, A_i)  -- a [128, 2048] dense matrix.
(The per-factor einsum 'jk,bkl,li->bji' with row-major reshape of x is
exactly (kron(B_i, A_i) @ x_token); summing the 10 factors collapses the
whole adapter into one GEMM.)

Distribution: pure data-parallel over tokens, 8 NeuronCores, no
collectives. The host packs x (cast to bf16) into a layout where each
512-token chunk is one DMA with 16 KiB-contiguous per-partition reads and
the contraction axis on SBUF partitions; the device runs a K=2048 bf16
GEMM accumulating in f32 PSUM [128f, 512t], silu on ScalarE (bf16 out),
PE-mode transpose to [t, f], LayerNorm stats via bn_stats/bn_aggr on
VectorE, and streams the normalized bf16 result back in a
partition-contiguous layout the host unpermutes.

Scheduling: software pipeline with a 1-chunk lag (chunk k's GEMM streams
on the PE while chunk k-1's transposes/stats run) so the PE never idles
waiting on the silu -- an idle PE drops the HAM clock to half rate. The
LayerNorm sqrt is batched per group of chunks so the ScalarE activation
table doesn't thrash between Silu and Sqrt on every chunk (~1.5 us per
swap), and the normalization applies are spread one chunk per iteration
so they never form a long serial burst in the VectorE stream.
"""

import os
import sys

import numpy as np

# --- problem constants (hardcoded; kernel.py must be self-contained) ---
N_CORES = 8
T_TOTAL = 4 * 8192
T_CORE = T_TOTAL // N_CORES  # 4096
N_IN = 2048
M_OUT = 128
CHUNK = 512                  # tokens per device-side pipeline step
NJ = CHUNK // 128            # 4 token sub-tiles per chunk
N_CHUNKS = T_CORE // CHUNK   # 8
GROUPS = [4, 2, 1, 1]        # chunks per batched-sqrt group
KC = N_IN // 128             # 16 contraction chunks
LN_EPS = 1e-5

_NC_CACHE = {}


def _ensure_paths():
    for p in ("/opt/trn_rl_repo",):
        if os.path.isdir(p) and p not in sys.path:
            sys.path.append(p)


def _install_ntff_hook():
    """Provide antenv.axon_hooks so trace=True can capture NTFF profiles
    under axon. Harmless if tracing is never requested."""
    import types

    try:
        import antenv
    except ImportError:
        return
    if "antenv.axon_hooks" in sys.modules:
        return
    mod = types.ModuleType("antenv.axon_hooks")
    state = {"hook": None, "tried": False}

    def get_axon_ntff_profile_hook():
        if state["hook"] is None and not state["tried"]:
            state["tried"] = True
            try:
                from trn_agent_boot.trn_boot import _ntff_profile_via_ctypes

                state["hook"] = _ntff_profile_via_ctypes("/opt/axon/libaxon_pjrt.so")
            except Exception:
                state["hook"] = None
        return state["hook"]

    mod.get_axon_ntff_profile_hook = get_axon_ntff_profile_hook
    mod.set_axon_ntff_profile_hook = lambda h: state.update(hook=h, tried=True)
    sys.modules["antenv.axon_hooks"] = mod
    antenv.axon_hooks = mod


def _build_nc(trivial_affine):
    import concourse.bass as bass  # noqa: F401
    import concourse.mybir as mybir
    import concourse.tile as tile
    from concourse import bacc

    F32 = mybir.dt.float32
    BF16 = mybir.dt.bfloat16
    AF = mybir.ActivationFunctionType
    ALU = mybir.AluOpType

    assert sum(GROUPS) == N_CHUNKS
    # chunk -> (group index, offset in group); group -> first chunk
    chunk_group = {}
    group_start = []
    c0 = 0
    for gi, gs in enumerate(GROUPS):
        group_start.append(c0)
        for kk in range(gs):
            chunk_group[c0 + kk] = (gi, kk)
        c0 += gs

    nc = bacc.Bacc()
    # host-packed: xt[k, p, h, t] = x[k*512 + t, h*128 + p], bf16
    xt_ext = nc.declare_dram_parameter(
        "xt", [N_CHUNKS, 128, KC, CHUNK], BF16, isOutput=False
    )
    # wt[p, h, f] = W[f, h*128 + p], bf16
    wt_ext = nc.declare_dram_parameter("wt", [128, KC, M_OUT], BF16, isOutput=False)
    id_ext = nc.declare_dram_parameter("ident", [128, 128], F32, isOutput=False)
    if not trivial_affine:
        gb_ext = nc.declare_dram_parameter("gb", [2, 128, 128], F32, isOutput=False)
    # out[k, p, j, f] = result[k*512 + j*128 + p, f]
    out_ext = nc.declare_dram_parameter(
        "out", [N_CHUNKS, 128, NJ, M_OUT], BF16, isOutput=True
    )

    with tile.TileContext(nc) as tc:
        with (
            tc.tile_pool(name="const", bufs=1) as constp,
            tc.tile_pool(name="xin", bufs=4) as xin,
            tc.tile_pool(name="work", bufs=4) as work,
            tc.tile_pool(name="outp", bufs=4) as outpool,
            tc.tile_pool(name="stats", bufs=2) as statsp,
            tc.tile_pool(name="pacc", bufs=2, space="PSUM") as pacc,
            tc.tile_pool(name="ptp", bufs=6, space="PSUM") as ptp,
        ):
            # Weights first (small, needed by the very first matmul), then
            # chunk 0's input quartered so the first GEMM starts after ~1/4
            # of the transfer instead of the full 2 MiB.
            wtt = constp.tile([128, KC, M_OUT], BF16)
            nc.sync.dma_start(out=wtt, in_=wt_ext[:, :, :])
            x0s = []
            for q in range(4):
                x0q = xin.tile([128, 4, CHUNK], BF16, tag=f"x0_{q}", name=f"x0_{q}")
                nc.sync.dma_start(out=x0q, in_=xt_ext[0][:, 4 * q:4 * q + 4, :])
                x0s.append(x0q)
            ident = constp.tile([128, 128], F32)
            nc.sync.dma_start(out=ident, in_=id_ext[:, :])
            if not trivial_affine:
                gbt = constp.tile([128, 2, 128], F32)
                nc.sync.dma_start(out=gbt, in_=gb_ext.rearrange("g p f -> p g f"))
            epst = constp.tile([128, 1], F32)
            nc.vector.memset(epst, float(LN_EPS))

            acts = {}    # chunk -> silu output tile
            tps = {}     # chunk -> transposed [t, f] PSUM tile
            mvgs = {}    # group -> mean/var tile
            rstds = {}   # group -> rstd tile
            apply_q = []  # chunks whose rstd is ready, awaiting apply

            def stats_stage(k):
                """Transposes + LN stats for chunk k; batched sqrt at group
                end (ScalarE table loads stay rare)."""
                g, kk = chunk_group[k]
                gs = GROUPS[g]
                if kk == 0:
                    mvgs[g] = statsp.tile(
                        [128, gs * NJ, 2], F32, tag="mv", name=f"mvg{g}"
                    )
                mvg = mvgs[g]
                tp = ptp.tile([128, CHUNK], F32, tag="tp", name=f"tp{k}")
                tps[k] = tp
                act = acts.pop(k)
                # All transposes first, then all stats: Tile's bank-level dep
                # tracking serializes same-bank PE-writes against DVE-reads in
                # emission order, so interleaving would stall each transpose
                # behind the previous bn_stats (~390 ns x 3 per chunk).
                for j in range(NJ):
                    fsl = slice(j * 128, (j + 1) * 128)
                    nc.tensor.transpose(tp[:, fsl], act[:, fsl], ident)
                for j in range(NJ):
                    fsl = slice(j * 128, (j + 1) * 128)
                    st = statsp.tile([128, 6], F32, tag="st")
                    nc.vector.bn_stats(out=st, in_=tp[:, fsl])
                    nc.vector.bn_aggr(out=mvg[:, kk * NJ + j, :], in_=st)

                if kk == gs - 1:
                    rstd = statsp.tile(
                        [128, gs * NJ], F32, tag="rstd", name=f"rstd{g}"
                    )
                    nc.scalar.activation(
                        out=rstd, in_=mvg[:, :, 1], func=AF.Sqrt, bias=epst
                    )
                    nc.vector.reciprocal(out=rstd, in_=rstd)
                    # Dummy op to pull the Silu-table reload off the critical
                    # path: it runs while the PE is still on the next GEMM,
                    # so the next real silu doesn't wait ~1.3us for the table.
                    # Reading mvg anchors it AFTER the sqrt in the ScalarE
                    # stream (a constant input would let the scheduler hoist
                    # it to kernel start; rstd would make it wait on the
                    # VectorE reciprocal and block the ScalarE stream).
                    dummy = statsp.tile([128, 1], F32, tag="dummy")
                    nc.scalar.activation(out=dummy, in_=mvg[:, 0, 0:1], func=AF.Silu)
                    rstds[g] = rstd
                    apply_q.extend(group_start[g] + i for i in range(gs))

            def apply_stage(k2):
                """Normalize chunk k2 from its PSUM transpose and DMA out."""
                g, kk2 = chunk_group[k2]
                mvg, rstd = mvgs[g], rstds[g]
                outsb = outpool.tile([128, NJ, 128], BF16, tag="outsb")
                for j in range(NJ):
                    fsl = slice(j * 128, (j + 1) * 128)
                    c = kk2 * NJ + j
                    if trivial_affine:
                        nc.vector.tensor_scalar(
                            out=outsb[:, j, :], in0=tps[k2][:, fsl],
                            scalar1=mvg[:, c, 0:1], scalar2=rstd[:, c:c + 1],
                            op0=ALU.subtract, op1=ALU.mult,
                        )
                    else:
                        z = statsp.tile([128, 128], F32, tag="z")
                        nc.vector.tensor_scalar(
                            out=z, in0=tps[k2][:, fsl],
                            scalar1=mvg[:, c, 0:1], scalar2=rstd[:, c:c + 1],
                            op0=ALU.subtract, op1=ALU.mult,
                        )
                        nc.vector.tensor_mul(out=z, in0=z, in1=gbt[:, 0, :])
                        nc.vector.tensor_add(
                            out=outsb[:, j, :], in0=z, in1=gbt[:, 1, :]
                        )
                tps.pop(k2)
                nc.sync.dma_start(out=out_ext[k2], in_=outsb)

            LAG = 2  # chunks of slack between a GEMM and its epilogue
            k = 0
            while k < N_CHUNKS + LAG or apply_q:
                if k < N_CHUNKS:
                    if k == 0:
                        rhs_of = lambda c: x0s[c // 4][:, c % 4, :]
                    else:
                        xtt = xin.tile([128, KC, CHUNK], BF16, tag="xt")
                        nc.sync.dma_start(out=xtt, in_=xt_ext[k])
                        rhs_of = lambda c: xtt[:, c, :]
                    acc = pacc.tile([128, CHUNK], F32)  # [f, t] in PSUM
                    for c in range(KC):
                        nc.tensor.matmul(
                            acc, lhsT=wtt[:, c, :], rhs=rhs_of(c),
                            start=(c == 0), stop=(c == KC - 1),
                        )
                    act = work.tile([128, CHUNK], F32, tag="act")
                    nc.scalar.activation(out=act, in_=acc, func=AF.Silu)
                    acts[k] = act
                if LAG <= k < N_CHUNKS + LAG:
                    stats_stage(k - LAG)
                if apply_q:
                    apply_stage(apply_q.pop(0))
                if apply_q and k >= N_CHUNKS - 1:
                    # GEMMs are winding down: drain applies faster so the
                    # normalization tail doesn't extend past the last stats.
                    apply_stage(apply_q.pop(0))
                k += 1

    nc.compile()
    return nc


def _host_weights(A_factors, B_factors, gate):
    gate = np.asarray(gate, dtype=np.float64)
    e = np.exp(gate - gate.max())
    alpha = e / e.sum()
    W = np.zeros((M_OUT, N_IN), dtype=np.float64)
    for i, (a, b) in enumerate(zip(A_factors, B_factors)):
        W += alpha[i] * np.kron(
            np.asarray(b, dtype=np.float64), np.asarray(a, dtype=np.float64)
        )
    return W  # [128, 2048] float64


def kernel(x, A_factors, B_factors, gate, ln_gamma, ln_beta):
    _ensure_paths()
    _install_ntff_hook()
    import ml_dtypes
    from concourse.bass_utils import run_bass_kernel_spmd

    BF = ml_dtypes.bfloat16

    x = np.asarray(x, dtype=np.float32)
    gamma = np.asarray(ln_gamma, dtype=np.float32).reshape(M_OUT)
    beta = np.asarray(ln_beta, dtype=np.float32).reshape(M_OUT)
    trivial = bool(np.all(gamma == 1.0) and np.all(beta == 0.0))

    if trivial not in _NC_CACHE:
        _NC_CACHE[trivial] = _build_nc(trivial)
    nc = _NC_CACHE[trivial]

    W = _host_weights(A_factors, B_factors, gate)  # [128, 2048] f64
    # wt[p, h, f] = W[f, h*128+p]
    wt = np.ascontiguousarray(
        W.T.reshape(KC, 128, M_OUT).transpose(1, 0, 2)
    ).astype(BF)
    ident = np.eye(128, dtype=np.float32)

    in_maps = []
    xs = x.reshape(N_CORES, N_CHUNKS, CHUNK, KC, 128)  # [c, k, t, h, p]
    for c in range(N_CORES):
        # xt[k, p, h, t] = x[c, k, t, h, p]
        xt = np.ascontiguousarray(xs[c].transpose(0, 3, 2, 1)).astype(BF)
        m = {"xt": xt, "wt": wt, "ident": ident}
        if not trivial:
            m["gb"] = np.ascontiguousarray(
                np.stack([
                    np.broadcast_to(gamma.reshape(1, M_OUT), (128, M_OUT)),
                    np.broadcast_to(beta.reshape(1, M_OUT), (128, M_OUT)),
                ])
            )
        in_maps.append(m)

    trace = bool(int(os.environ.get("ADAPTER_TRACE", "0")))
    kwargs = {}
    if trace:
        kwargs["trace"] = True
        tdir = os.environ.get("ADAPTER_TRACE_DIR")
        if tdir:
            os.makedirs(tdir, exist_ok=True)
            kwargs["tmpdir"] = tdir

    res = None
    last_err = None
    for _attempt in range(3):
        try:
            res = run_bass_kernel_spmd(
                nc, in_maps, core_ids=list(range(N_CORES)), **kwargs
            )
            break
        except Exception as e:  # transient device wedge -> retry
            last_err = e
    if res is None:
        raise last_err
    if trace:
        print(f"HW exec time: {res.exec_time_ns} ns")

    # out[k, p, j, f] -> tokens k*512 + j*128 + p
    outs = [
        res.results[c]["out"].astype(np.float32)
        .transpose(0, 2, 1, 3).reshape(T_CORE, M_OUT)
        for c in range(N_CORES)
    ]
    out = np.concatenate(outs, axis=0)
    return out.reshape(4, 8192, M_OUT).astype(np.float32)


# revision 24
# speedup vs baseline: 1.1618x; 1.0083x over previous
"""Trainium2 Bass kernel for nn_AdapterModule_46050639348236.

Math: out = LayerNorm(silu(x @ W^T)) * gamma + beta, where
W = sum_i softmax(gate)_i * kron(B_i, A_i)  -- a [128, 2048] dense matrix.
(The per-factor einsum 'jk,bkl,li->bji' with row-major reshape of x is
exactly (kron(B_i, A_i) @ x_token); summing the 10 factors collapses the
whole adapter into one GEMM.)

Distribution: pure data-parallel over tokens, 8 NeuronCores, no
collectives. The host packs x (cast to bf16) into a layout where each
512-token chunk is one DMA with 16 KiB-contiguous per-partition reads and
the contraction axis on SBUF partitions; the device runs a K=2048 bf16
GEMM accumulating in f32 PSUM [128f, 512t], silu on ScalarE (bf16 out),
PE-mode transpose to [t, f], LayerNorm stats via bn_stats/bn_aggr on
VectorE, and streams the normalized bf16 result back in a
partition-contiguous layout the host unpermutes.

Scheduling: software pipeline with a 1-chunk lag (chunk k's GEMM streams
on the PE while chunk k-1's transposes/stats run) so the PE never idles
waiting on the silu -- an idle PE drops the HAM clock to half rate. The
LayerNorm sqrt is batched per group of chunks so the ScalarE activation
table doesn't thrash between Silu and Sqrt on every chunk (~1.5 us per
swap), and the normalization applies are spread one chunk per iteration
so they never form a long serial burst in the VectorE stream.
"""

import os
import sys

import numpy as np

# --- problem constants (hardcoded; kernel.py must be self-contained) ---
N_CORES = 8
T_TOTAL = 4 * 8192
T_CORE = T_TOTAL // N_CORES  # 4096
N_IN = 2048
M_OUT = 128
CHUNK = 512                  # tokens per device-side pipeline step
NJ = CHUNK // 128            # 4 token sub-tiles per chunk
N_CHUNKS = T_CORE // CHUNK   # 8
GROUPS = [4, 2, 1, 1]        # chunks per batched-sqrt group
KC = N_IN // 128             # 16 contraction chunks
LN_EPS = 1e-5

_NC_CACHE = {}


def _ensure_paths():
    for p in ("/opt/trn_rl_repo",):
        if os.path.isdir(p) and p not in sys.path:
            sys.path.append(p)


def _install_ntff_hook():
    """Provide antenv.axon_hooks so trace=True can capture NTFF profiles
    under axon. Harmless if tracing is never requested."""
    import types

    try:
        import antenv
    except ImportError:
        return
    if "antenv.axon_hooks" in sys.modules:
        return
    mod = types.ModuleType("antenv.axon_hooks")
    state = {"hook": None, "tried": False}

    def get_axon_ntff_profile_hook():
        if state["hook"] is None and not state["tried"]:
            state["tried"] = True
            try:
                from trn_agent_boot.trn_boot import _ntff_profile_via_ctypes

                state["hook"] = _ntff_profile_via_ctypes("/opt/axon/libaxon_pjrt.so")
            except Exception:
                state["hook"] = None
        return state["hook"]

    mod.get_axon_ntff_profile_hook = get_axon_ntff_profile_hook
    mod.set_axon_ntff_profile_hook = lambda h: state.update(hook=h, tried=True)
    sys.modules["antenv.axon_hooks"] = mod
    antenv.axon_hooks = mod


def _build_nc(trivial_affine):
    import concourse.bass as bass  # noqa: F401
    import concourse.mybir as mybir
    import concourse.tile as tile
    from concourse import bacc

    F32 = mybir.dt.float32
    BF16 = mybir.dt.bfloat16
    AF = mybir.ActivationFunctionType
    ALU = mybir.AluOpType

    assert sum(GROUPS) == N_CHUNKS
    # chunk -> (group index, offset in group); group -> first chunk
    chunk_group = {}
    group_start = []
    c0 = 0
    for gi, gs in enumerate(GROUPS):
        group_start.append(c0)
        for kk in range(gs):
            chunk_group[c0 + kk] = (gi, kk)
        c0 += gs

    nc = bacc.Bacc()
    # host-packed: xt[k, p, h, t] = x[k*512 + t, h*128 + p], bf16
    xt_ext = nc.declare_dram_parameter(
        "xt", [N_CHUNKS, 128, KC, CHUNK], BF16, isOutput=False
    )
    # wt[p, h, f] = W[f, h*128 + p], bf16
    wt_ext = nc.declare_dram_parameter("wt", [128, KC, M_OUT], BF16, isOutput=False)
    id_ext = nc.declare_dram_parameter("ident", [128, 128], F32, isOutput=False)
    if not trivial_affine:
        gb_ext = nc.declare_dram_parameter("gb", [2, 128, 128], F32, isOutput=False)
    # out[k, p, j, f] = result[k*512 + j*128 + p, f]
    out_ext = nc.declare_dram_parameter(
        "out", [N_CHUNKS, 128, NJ, M_OUT], BF16, isOutput=True
    )

    with tile.TileContext(nc) as tc:
        with (
            tc.tile_pool(name="const", bufs=1) as constp,
            tc.tile_pool(name="xin", bufs=4) as xin,
            tc.tile_pool(name="work", bufs=4) as work,
            tc.tile_pool(name="outp", bufs=4) as outpool,
            tc.tile_pool(name="stats", bufs=2) as statsp,
            tc.tile_pool(name="pacc", bufs=2, space="PSUM") as pacc,
            tc.tile_pool(name="ptp", bufs=6, space="PSUM") as ptp,
        ):
            # Weights first (small, needed by the very first matmul), then
            # chunk 0's input quartered so the first GEMM starts after ~1/4
            # of the transfer instead of the full 2 MiB.
            wtt = constp.tile([128, KC, M_OUT], BF16)
            nc.sync.dma_start(out=wtt, in_=wt_ext[:, :, :])
            x0s = []
            for q in range(4):
                x0q = xin.tile([128, 4, CHUNK], BF16, tag=f"x0_{q}", name=f"x0_{q}")
                nc.sync.dma_start(out=x0q, in_=xt_ext[0][:, 4 * q:4 * q + 4, :])
                x0s.append(x0q)
            ident = constp.tile([128, 128], F32)
            nc.sync.dma_start(out=ident, in_=id_ext[:, :])
            if not trivial_affine:
                gbt = constp.tile([128, 2, 128], F32)
                nc.sync.dma_start(out=gbt, in_=gb_ext.rearrange("g p f -> p g f"))
            epst = constp.tile([128, 1], F32)
            nc.vector.memset(epst, float(LN_EPS))

            acts = {}    # chunk -> silu output tile
            tps = {}     # chunk -> transposed [t, f] PSUM tile
            mvgs = {}    # group -> mean/var tile
            rstds = {}   # group -> rstd tile
            apply_q = []  # chunks whose rstd is ready, awaiting apply

            def stats_stage(k):
                """Transposes + LN stats for chunk k; batched sqrt at group
                end (ScalarE table loads stay rare)."""
                g, kk = chunk_group[k]
                gs = GROUPS[g]
                if kk == 0:
                    mvgs[g] = statsp.tile(
                        [128, gs * NJ, 2], F32, tag="mv", name=f"mvg{g}"
                    )
                mvg = mvgs[g]
                tp = ptp.tile([128, CHUNK], F32, tag="tp", name=f"tp{k}")
                tps[k] = tp
                act = acts.pop(k)
                # All transposes first, then all stats: Tile's bank-level dep
                # tracking serializes same-bank PE-writes against DVE-reads in
                # emission order, so interleaving would stall each transpose
                # behind the previous bn_stats (~390 ns x 3 per chunk).
                for j in range(NJ):
                    fsl = slice(j * 128, (j + 1) * 128)
                    nc.tensor.transpose(tp[:, fsl], act[:, fsl], ident)
                for j in range(NJ):
                    fsl = slice(j * 128, (j + 1) * 128)
                    st = statsp.tile([128, 6], F32, tag="st")
                    nc.vector.bn_stats(out=st, in_=tp[:, fsl])
                    nc.vector.bn_aggr(out=mvg[:, kk * NJ + j, :], in_=st)

                if kk == gs - 1:
                    std = statsp.tile([128, gs * NJ], F32, tag="std", name=f"std{g}")
                    nc.scalar.activation(
                        out=std, in_=mvg[:, :, 1], func=AF.Sqrt, bias=epst
                    )
                    rstd = statsp.tile(
                        [128, gs * NJ], F32, tag="rstd", name=f"rstd{g}"
                    )
                    nc.vector.reciprocal(out=rstd, in_=std)
                    # Dummy op to pull the Silu-table reload off the critical
                    # path: it runs right after the sqrt on ScalarE (while the
                    # reciprocal proceeds on VectorE), so the next real silu
                    # doesn't wait ~1.3us for the table. It must read the
                    # sqrt's own output: a constant input gets hoisted to
                    # kernel start, an mvg slice fires after the group's first
                    # bn_aggr (range-level deps), and rstd would block ScalarE
                    # on the VectorE reciprocal.
                    dummy = statsp.tile([128, 1], F32, tag="dummy")
                    nc.scalar.activation(out=dummy, in_=std[:, 0:1], func=AF.Silu)
                    rstds[g] = rstd
                    apply_q.extend(group_start[g] + i for i in range(gs))

            def apply_stage(k2):
                """Normalize chunk k2 from its PSUM transpose and DMA out."""
                g, kk2 = chunk_group[k2]
                mvg, rstd = mvgs[g], rstds[g]
                outsb = outpool.tile([128, NJ, 128], BF16, tag="outsb")
                for j in range(NJ):
                    fsl = slice(j * 128, (j + 1) * 128)
                    c = kk2 * NJ + j
                    if trivial_affine:
                        nc.vector.tensor_scalar(
                            out=outsb[:, j, :], in0=tps[k2][:, fsl],
                            scalar1=mvg[:, c, 0:1], scalar2=rstd[:, c:c + 1],
                            op0=ALU.subtract, op1=ALU.mult,
                        )
                    else:
                        z = statsp.tile([128, 128], F32, tag="z")
                        nc.vector.tensor_scalar(
                            out=z, in0=tps[k2][:, fsl],
                            scalar1=mvg[:, c, 0:1], scalar2=rstd[:, c:c + 1],
                            op0=ALU.subtract, op1=ALU.mult,
                        )
                        nc.vector.tensor_mul(out=z, in0=z, in1=gbt[:, 0, :])
                        nc.vector.tensor_add(
                            out=outsb[:, j, :], in0=z, in1=gbt[:, 1, :]
                        )
                tps.pop(k2)
                nc.sync.dma_start(out=out_ext[k2], in_=outsb)

            LAG = 2  # chunks of slack between a GEMM and its epilogue
            k = 0
            while k < N_CHUNKS + LAG or apply_q:
                if k < N_CHUNKS:
                    if k == 0:
                        rhs_of = lambda c: x0s[c // 4][:, c % 4, :]
                    else:
                        xtt = xin.tile([128, KC, CHUNK], BF16, tag="xt")
                        nc.sync.dma_start(out=xtt, in_=xt_ext[k])
                        rhs_of = lambda c: xtt[:, c, :]
                    acc = pacc.tile([128, CHUNK], F32)  # [f, t] in PSUM
                    for c in range(KC):
                        nc.tensor.matmul(
                            acc, lhsT=wtt[:, c, :], rhs=rhs_of(c),
                            start=(c == 0), stop=(c == KC - 1),
                        )
                    act = work.tile([128, CHUNK], F32, tag="act")
                    nc.scalar.activation(out=act, in_=acc, func=AF.Silu)
                    acts[k] = act
                if LAG <= k < N_CHUNKS + LAG:
                    stats_stage(k - LAG)
                if apply_q:
                    apply_stage(apply_q.pop(0))
                if apply_q and k >= N_CHUNKS - 1:
                    # GEMMs are winding down: drain applies faster so the
                    # normalization tail doesn't extend past the last stats.
                    apply_stage(apply_q.pop(0))
                k += 1

    nc.compile()
    return nc


def _host_weights(A_factors, B_factors, gate):
    gate = np.asarray(gate, dtype=np.float64)
    e = np.exp(gate - gate.max())
    alpha = e / e.sum()
    W = np.zeros((M_OUT, N_IN), dtype=np.float64)
    for i, (a, b) in enumerate(zip(A_factors, B_factors)):
        W += alpha[i] * np.kron(
            np.asarray(b, dtype=np.float64), np.asarray(a, dtype=np.float64)
        )
    return W  # [128, 2048] float64


def kernel(x, A_factors, B_factors, gate, ln_gamma, ln_beta):
    _ensure_paths()
    _install_ntff_hook()
    import ml_dtypes
    from concourse.bass_utils import run_bass_kernel_spmd

    BF = ml_dtypes.bfloat16

    x = np.asarray(x, dtype=np.float32)
    gamma = np.asarray(ln_gamma, dtype=np.float32).reshape(M_OUT)
    beta = np.asarray(ln_beta, dtype=np.float32).reshape(M_OUT)
    trivial = bool(np.all(gamma == 1.0) and np.all(beta == 0.0))

    if trivial not in _NC_CACHE:
        _NC_CACHE[trivial] = _build_nc(trivial)
    nc = _NC_CACHE[trivial]

    W = _host_weights(A_factors, B_factors, gate)  # [128, 2048] f64
    # wt[p, h, f] = W[f, h*128+p]
    wt = np.ascontiguousarray(
        W.T.reshape(KC, 128, M_OUT).transpose(1, 0, 2)
    ).astype(BF)
    ident = np.eye(128, dtype=np.float32)

    in_maps = []
    xs = x.reshape(N_CORES, N_CHUNKS, CHUNK, KC, 128)  # [c, k, t, h, p]
    for c in range(N_CORES):
        # xt[k, p, h, t] = x[c, k, t, h, p]
        xt = np.ascontiguousarray(xs[c].transpose(0, 3, 2, 1)).astype(BF)
        m = {"xt": xt, "wt": wt, "ident": ident}
        if not trivial:
            m["gb"] = np.ascontiguousarray(
                np.stack([
                    np.broadcast_to(gamma.reshape(1, M_OUT), (128, M_OUT)),
                    np.broadcast_to(beta.reshape(1, M_OUT), (128, M_OUT)),
                ])
            )
        in_maps.append(m)

    trace = bool(int(os.environ.get("ADAPTER_TRACE", "0")))
    kwargs = {}
    if trace:
        kwargs["trace"] = True
        tdir = os.environ.get("ADAPTER_TRACE_DIR")
        if tdir:
            os.makedirs(tdir, exist_ok=True)
            kwargs["tmpdir"] = tdir

    res = None
    last_err = None
    for _attempt in range(3):
        try:
            res = run_bass_kernel_spmd(
                nc, in_maps, core_ids=list(range(N_CORES)), **kwargs
            )
            break
        except Exception as e:  # transient device wedge -> retry
            last_err = e
    if res is None:
        raise last_err
    if trace:
        print(f"HW exec time: {res.exec_time_ns} ns")

    # out[k, p, j, f] -> tokens k*512 + j*128 + p
    outs = [
        res.results[c]["out"].astype(np.float32)
        .transpose(0, 2, 1, 3).reshape(T_CORE, M_OUT)
        for c in range(N_CORES)
    ]
    out = np.concatenate(outs, axis=0)
    return out.reshape(4, 8192, M_OUT).astype(np.float32)
